# revision 1
# baseline (speedup 1.0000x reference)
"""Trainium2 Bass kernel for a 2-layer edge-weighted GCN (gnn_message_passing).

Math (matches reference.py):
    w_e   = softplus(edge_feats @ We + be)            per edge
    deg_d = sum_{e: dst=d} w_e + 1                    (self loop w=1)
    dinv  = 1/sqrt(deg)
    A     = D^-1/2 (W_adj + I) D^-1/2
    x1    = tanh((A @ X) @ W1 + b1)
    out   = (A @ x1) @ W2 + b2 + X @ Ws + bs

Distribution: edges sorted by dst, sharded across 8 cores at node
boundaries (each core owns a contiguous node range => its scatter-adds
are complete locally, no all-reduce).  Two small AllGathers move dinv
and the scaled hidden state x1s = dinv*x1 so every core can gather
arbitrary source rows.

On-core scatter-add: edges are packed into groups of F tiles x 128
edges whose dst values all fall in one 128-node window.  Per tile a
[128,128] weighted one-hot matrix P (P[j,i] = w_j * (dstoff_j == i)) is
built with one DVE tensor_scalar op, and PSUM accumulates
P^T-matmuls: agg[i,f] += sum_j P[j,i] * gathered[j,f].
"""

import os
import sys

import numpy as np

for _p in ("/opt/trn_rl_repo",):
    if _p not in sys.path and os.path.isdir(_p):
        sys.path.insert(0, _p)

# ---------------- problem constants (hardcoded per spec) ----------------
N_NODES = 50000
N_EDGES = 800000
D_EDGE = 8
D_IN = 128
D_HID = 256
D_OUT = 256
NCORES = 8
P = 128
F = 16  # tiles (of 128 edges) per group / 128-node window


# ======================================================================
# Host-side preprocessing
# ======================================================================

def _preprocess(edge_index, n_nodes=N_NODES, ncores=NCORES, f_tiles=F):
    src = np.asarray(edge_index[0]).astype(np.int64)
    dst = np.asarray(edge_index[1]).astype(np.int64)
    E = src.shape[0]

    order = np.argsort(dst, kind="stable")
    src_s = src[order]
    dst_s = dst[order]

    cnt = np.bincount(dst_s, minlength=n_nodes).astype(np.int64)
    cum = np.cumsum(cnt)  # edges with dst <= n

    # node range cuts with ~equal edge counts
    node_lo = [0]
    for k in range(1, ncores):
        t = k * E // ncores
        n = int(np.searchsorted(cum, t))
        node_lo.append(max(node_lo[-1] + 1, min(n + 1, n_nodes - (ncores - k))))
    node_lo.append(n_nodes)

    cap = f_tiles * P
    cores = []
    for k in range(ncores):
        lo, hi = node_lo[k], node_lo[k + 1]
        groups = []  # (base, node_end, e_start, e_end)
        base = lo
        e_start = int(cum[lo - 1]) if lo > 0 else 0
        cur = 0
        for n in range(lo, hi):
            c = int(cnt[n])
            assert c <= cap, f"node {n} has {c} edges > cap {cap}"
            if (n - base >= P) or (cur + c > cap):
                groups.append((base, n, e_start, e_start + cur))
                base = n
                e_start = e_start + cur
                cur = 0
            cur += c
        groups.append((base, hi, e_start, e_start + cur))
        cores.append({"lo": lo, "hi": hi, "groups": groups})

    NG = max(len(c["groups"]) for c in cores)
    T = NG * f_tiles

    # node -> slab row LUT (slab row = core*NG*128 + g*128 + (n - base_g))
    lut = np.zeros(n_nodes, dtype=np.int64)
    for k, c in enumerate(cores):
        for g, (base, nend, _, _) in enumerate(c["groups"]):
            if nend > base:
                lut[base:nend] = k * NG * P + g * P + (np.arange(base, nend) - base)

    # dinv slab mapping: dinv stored as [128, NG] per core (partition-major):
    # node n (core k, group g, offset p) -> k*128*NG + p*NG + g
    dinv_lut = np.zeros(n_nodes, dtype=np.int64)
    for k, c in enumerate(cores):
        for g, (base, nend, _, _) in enumerate(c["groups"]):
            if nend > base:
                p = np.arange(base, nend) - base
                dinv_lut[base:nend] = k * P * NG + p * NG + g

    per_core = []
    for k, c in enumerate(cores):
        srcq = np.zeros((P, T), dtype=np.int32)
        src2q = np.zeros((P, T), dtype=np.int32)  # x1s slab row of src
        sdivq = np.zeros((P, T), dtype=np.int32)  # dinv slab row of src
        dstoff = np.zeros((P, T), dtype=np.float32)
        wmask = np.zeros((P, T), dtype=np.float32)
        eperm = np.full((P, T), -1, dtype=np.int64)  # global (sorted) edge id per slot

        for g, (base, nend, e0, e1) in enumerate(c["groups"]):
            ne = e1 - e0
            if ne == 0:
                continue
            ids = np.arange(e0, e1)
            t_idx = (ids - e0) // P
            p_idx = (ids - e0) % P
            colv = g * f_tiles + t_idx
            srcq[p_idx, colv] = src_s[ids].astype(np.int32)
            src2q[p_idx, colv] = lut[src_s[ids]].astype(np.int32)
            sdivq[p_idx, colv] = dinv_lut[src_s[ids]].astype(np.int32)
            dstoff[p_idx, colv] = (dst_s[ids] - base).astype(np.float32)
            wmask[p_idx, colv] = 1.0
            eperm[p_idx, colv] = ids

        per_core.append(
            {
                "srcq": srcq,
                "src2q": src2q,
                "sdivq": sdivq,
                "dstoff": dstoff,
                "wmask": wmask,
                "eperm": eperm,
                "lo": c["lo"],
                "hi": c["hi"],
                "groups": c["groups"],
            }
        )

    return {"NG": NG, "T": T, "order": order, "per_core": per_core}


def _host_inputs(prep, edge_feats, node_feats):
    """Build the numpy in_map for every core."""
    NG, T = prep["NG"], prep["T"]
    ef_s = np.asarray(edge_feats, dtype=np.float32)[prep["order"]]
    X = np.ascontiguousarray(np.asarray(node_feats, dtype=np.float32))

    maps = []
    for pc in prep["per_core"]:
        efq = np.zeros((P, D_EDGE * T), dtype=np.float32)
        valid = pc["eperm"] >= 0
        pv, cv = np.nonzero(valid)
        eids = pc["eperm"][pv, cv]
        for kf in range(D_EDGE):
            efq[pv, kf * T + cv] = ef_s[eids, kf]

        xwin = np.zeros((NG * P, D_IN), dtype=np.float32)
        for g, (base, nend, _, _) in enumerate(pc["groups"]):
            hi = min(base + P, N_NODES)
            xwin[g * P : g * P + (hi - base)] = X[base:hi]
        xwT = np.ascontiguousarray(
            xwin.reshape(NG, P, D_IN).transpose(2, 0, 1).reshape(D_IN, NG * P)
        )

        maps.append(
            {
                "srcq": pc["srcq"],
                "src2q": pc["src2q"],
                "sdivq": pc["sdivq"],
                "dstoff": pc["dstoff"],
                "wmask": pc["wmask"],
                "efq": efq,
                "xwin": xwin,
                "xwinT": xwT,
            }
        )
    return maps


# ======================================================================
# Bass program
# ======================================================================

def _build_program(NG, n_nodes=N_NODES, debug=False):
    import concourse.bacc as bacc
    import concourse.bass as bass
    import concourse.mybir as mybir
    from concourse.masks import make_identity
    from concourse.tile import TileContext

    f32 = mybir.dt.float32
    i32 = mybir.dt.int32
    AF = mybir.ActivationFunctionType
    OP = mybir.AluOpType
    T = NG * F

    nc = bacc.Bacc(trn_type="TRN2", num_devices=NCORES)

    # ---- I/O ----
    src2q = nc.dram_tensor("src2q", [P, T], i32, kind="ExternalInput")
    dstoff = nc.dram_tensor("dstoff", [P, T], f32, kind="ExternalInput")
    wmask = nc.dram_tensor("wmask", [P, T], f32, kind="ExternalInput")
    efq = nc.dram_tensor("efq", [P, D_EDGE * T], f32, kind="ExternalInput")
    xwin = nc.dram_tensor("xwin", [NG * P, D_IN], f32, kind="ExternalInput")
    xwinT = nc.dram_tensor("xwinT", [D_IN, NG * P], f32, kind="ExternalInput")
    We_r = nc.dram_tensor("We_r", [1, D_EDGE], f32, kind="ExternalInput")
    be_r = nc.dram_tensor("be_r", [1, 1], f32, kind="ExternalInput")
    W1_t = nc.dram_tensor("W1", [D_IN, D_HID], f32, kind="ExternalInput")
    b1_r = nc.dram_tensor("b1_r", [1, D_HID], f32, kind="ExternalInput")
    W2_t = nc.dram_tensor("W2", [D_HID, D_OUT], f32, kind="ExternalInput")
    Ws_t = nc.dram_tensor("Ws", [D_IN, D_OUT], f32, kind="ExternalInput")
    b2s_r = nc.dram_tensor("b2s_r", [1, D_OUT], f32, kind="ExternalInput")

    g1_loc = nc.dram_tensor("g1_loc", [NG * P, D_IN], f32, kind="Internal")
    g1_full = nc.dram_tensor(
        "g1_full", [NCORES * NG * P, D_IN], f32, kind="Internal", addr_space="Shared"
    )
    x1s_loc = nc.dram_tensor("x1s_loc", [NG * P, D_HID], f32, kind="Internal")
    x1s_full = nc.dram_tensor(
        "x1s_full", [NCORES * NG * P, D_HID], f32, kind="Internal", addr_space="Shared"
    )
    out_loc = nc.dram_tensor("out_loc", [NG * P, D_OUT], f32, kind="ExternalOutput")
    if debug:
        dbg_w = nc.dram_tensor("dbg_w", [P, NG * F], f32, kind="ExternalOutput")
        dbg_dinv = nc.dram_tensor("dbg_dinv", [P, NG], f32, kind="ExternalOutput")
        dbg_x1s = nc.dram_tensor(
            "dbg_x1s", [P, NG * D_HID], f32, kind="ExternalOutput"
        )
        dbg_gat = nc.dram_tensor("dbg_gat", [P, F * D_IN], f32, kind="ExternalOutput")
        dbg_agg = nc.dram_tensor("dbg_agg", [P, D_IN], f32, kind="ExternalOutput")

    rg = [list(range(NCORES))]

    with TileContext(nc) as tc:
        with (
            tc.tile_pool(name="const", bufs=1) as cpool,
            tc.tile_pool(name="big", bufs=1) as bpool,
            tc.tile_pool(name="gat", bufs=2) as gpool,
            tc.tile_pool(name="ptw", bufs=4) as ppool,
            tc.tile_pool(name="eps", bufs=3) as epool,
            tc.tile_pool(name="psum", bufs=2, space="PSUM") as pspool,
        ):
            # ---------------- constants / weights ----------------
            iota_i = cpool.tile([P, P], dtype=i32)
            nc.gpsimd.iota(iota_i[:], pattern=[[1, P]], base=0, channel_multiplier=0)
            iota_f = cpool.tile([P, P], dtype=f32)
            nc.vector.tensor_copy(iota_f[:], iota_i[:])

            ident = cpool.tile([P, P], dtype=f32)
            make_identity(nc, ident[:])

            W1s = cpool.tile([D_IN, D_HID], dtype=f32)
            nc.sync.dma_start(out=W1s[:], in_=W1_t[:, :])
            W2a = cpool.tile([P, D_OUT], dtype=f32)
            nc.sync.dma_start(out=W2a[:], in_=W2_t[0:P, :])
            W2b = cpool.tile([P, D_OUT], dtype=f32)
            nc.sync.dma_start(out=W2b[:], in_=W2_t[P : 2 * P, :])
            Wss = cpool.tile([D_IN, D_OUT], dtype=f32)
            nc.sync.dma_start(out=Wss[:], in_=Ws_t[:, :])

            we_ld = cpool.tile([1, D_EDGE], dtype=f32)
            nc.sync.dma_start(out=we_ld[:], in_=We_r[:, :])
            WeB = cpool.tile([P, D_EDGE], dtype=f32)
            nc.gpsimd.partition_broadcast(WeB[:], we_ld[:1, :])

            be_ld = cpool.tile([1, 1], dtype=f32)
            nc.sync.dma_start(out=be_ld[:], in_=be_r[:, :])
            beB = cpool.tile([P, 1], dtype=f32)
            nc.gpsimd.partition_broadcast(beB[:], be_ld[:1, :])

            b1_ld = cpool.tile([1, D_HID], dtype=f32)
            nc.sync.dma_start(out=b1_ld[:], in_=b1_r[:, :])
            b1B = cpool.tile([P, D_HID], dtype=f32)
            nc.gpsimd.partition_broadcast(b1B[:], b1_ld[:1, :])

            b2s_ld = cpool.tile([1, D_OUT], dtype=f32)
            nc.sync.dma_start(out=b2s_ld[:], in_=b2s_r[:, :])
            b2sB = cpool.tile([P, D_OUT], dtype=f32)
            nc.gpsimd.partition_broadcast(b2sB[:], b2s_ld[:1, :])

            # ---------------- resident per-edge data ----------------
            src2T = bpool.tile([P, T], dtype=i32)
            nc.sync.dma_start(out=src2T[:], in_=src2q[:, :])
            dstT = bpool.tile([P, T], dtype=f32)
            nc.sync.dma_start(out=dstT[:], in_=dstoff[:, :])
            mskT = bpool.tile([P, T], dtype=f32)
            nc.sync.dma_start(out=mskT[:], in_=wmask[:, :])
            efT = bpool.tile([P, D_EDGE * T], dtype=f32)
            nc.sync.dma_start(out=efT[:], in_=efq[:, :])

            wT = bpool.tile([P, T], dtype=f32)
            x1sT = bpool.tile([P, NG * D_HID], dtype=f32)
            dinvT = bpool.tile([P, NG], dtype=f32)

            # ---------------- stage A: edge weights ----------------
            tmpA = bpool.tile([P, T], dtype=f32)
            tmpB = bpool.tile([P, T], dtype=f32)
            nc.vector.tensor_scalar(
                out=wT[:], in0=efT[:, 0:T], scalar1=WeB[:, 0:1], scalar2=None,
                op0=OP.mult,
            )
            for kf in range(1, D_EDGE):
                nc.vector.tensor_scalar(
                    out=tmpA[:], in0=efT[:, kf * T : (kf + 1) * T],
                    scalar1=WeB[:, kf : kf + 1], scalar2=None, op0=OP.mult,
                )
                nc.vector.tensor_tensor(out=wT[:], in0=wT[:], in1=tmpA[:], op=OP.add)
            nc.vector.tensor_scalar(
                out=wT[:], in0=wT[:], scalar1=beB[:, 0:1], scalar2=None, op0=OP.add
            )
            # softplus(x) = relu(x) + log(1 + exp(-|x|))
            nc.scalar.activation(out=tmpA[:], in_=wT[:], func=AF.Abs)
            nc.scalar.activation(out=tmpA[:], in_=tmpA[:], func=AF.Exp, scale=-1.0)
            nc.scalar.activation(out=tmpA[:], in_=tmpA[:], func=AF.Ln, bias=1.0)
            nc.scalar.activation(out=tmpB[:], in_=wT[:], func=AF.Relu)
            nc.vector.tensor_tensor(out=wT[:], in0=tmpA[:], in1=tmpB[:], op=OP.add)
            nc.vector.tensor_tensor(out=wT[:], in0=wT[:], in1=mskT[:], op=OP.mult)
            if debug:
                nc.sync.dma_start(out=dbg_w[:, :], in_=wT[:])

            # ---------------- stage B: degree + dinv ----------------
            for g in range(NG):
                degp = pspool.tile([P, P], dtype=f32, tag="small")
                for t in range(F):
                    col = g * F + t
                    pt = ppool.tile([P, P], dtype=f32, tag="pt")
                    nc.vector.tensor_scalar(
                        out=pt[:], in0=iota_f[:], scalar1=dstT[:, col : col + 1],
                        scalar2=None, op0=OP.is_equal,
                    )
                    nc.tensor.matmul(
                        degp[:, 0:1], lhsT=pt[:], rhs=wT[:, col : col + 1],
                        start=(t == 0), stop=(t == F - 1),
                    )
                # dinv = 1/sqrt(deg + 1)
                sq = epool.tile([P, 1], dtype=f32, tag="sq")
                nc.scalar.activation(
                    out=sq[:], in_=degp[:, 0:1], func=AF.Sqrt, bias=1.0
                )
                nc.vector.reciprocal(dinvT[:, g : g + 1], sq[:])
                # g1 = dinv * Xwin for owned rows -> g1_loc slab
                xwb = epool.tile([P, D_IN], dtype=f32, tag="xwb")
                nc.sync.dma_start(out=xwb[:], in_=xwin[g * P : (g + 1) * P, :])
                g1b = epool.tile([P, D_IN], dtype=f32, tag="g1b")
                nc.vector.tensor_scalar(
                    out=g1b[:], in0=xwb[:], scalar1=dinvT[:, g : g + 1],
                    scalar2=None, op0=OP.mult,
                )
                nc.sync.dma_start(out=g1_loc[g * P : (g + 1) * P, :], in_=g1b[:])
            if debug:
                nc.sync.dma_start(out=dbg_dinv[:, :], in_=dinvT[:])

            # ---------------- stage C: allgather g1 ----------------
            nc.gpsimd.collective_compute(
                "AllGather", OP.bypass, rg, ins=[g1_loc[:, :]],
                outs=[g1_full[:, :]],
            )

            # ---------------- stage E: layer 1 ----------------
            for g in range(NG):
                gat = gpool.tile([P, F * D_IN], dtype=f32, tag="gat")
                for t in range(F):
                    col = g * F + t
                    nc.gpsimd.indirect_dma_start(
                        out=gat[:, t * D_IN : (t + 1) * D_IN],
                        out_offset=None, in_=g1_full[:, :],
                        in_offset=bass.IndirectOffsetOnAxis(
                            ap=src2T[:, col : col + 1], axis=0
                        ),
                    )
                aggp = pspool.tile([P, D_HID], dtype=f32, tag="agg")
                for t in range(F):
                    col = g * F + t
                    pt = ppool.tile([P, P], dtype=f32, tag="pt")
                    nc.vector.tensor_scalar(
                        out=pt[:], in0=iota_f[:], scalar1=dstT[:, col : col + 1],
                        scalar2=wT[:, col : col + 1], op0=OP.is_equal, op1=OP.mult,
                    )
                    nc.tensor.matmul(
                        aggp[:, 0:D_IN], lhsT=pt[:],
                        rhs=gat[:, t * D_IN : (t + 1) * D_IN],
                        start=(t == 0), stop=(t == F - 1),
                    )
                if debug and g == 0:
                    nc.sync.dma_start(out=dbg_gat[:, :], in_=gat[:])
                    aggc = epool.tile([P, D_IN], dtype=f32, tag="aggc")
                    nc.vector.tensor_copy(aggc[:], aggp[:, 0:D_IN])
                    nc.sync.dma_start(out=dbg_agg[:, :], in_=aggc[:])
                # AX = dinv * (agg + dinv * Xwin)
                xw = epool.tile([P, D_IN], dtype=f32, tag="xw")
                nc.sync.dma_start(out=xw[:], in_=xwin[g * P : (g + 1) * P, :])
                t1 = epool.tile([P, D_IN], dtype=f32, tag="t1")
                nc.vector.tensor_scalar(
                    out=t1[:], in0=xw[:], scalar1=dinvT[:, g : g + 1], scalar2=None,
                    op0=OP.mult,
                )
                t2 = epool.tile([P, D_IN], dtype=f32, tag="t2")
                nc.vector.tensor_tensor(
                    out=t2[:], in0=aggp[:, 0:D_IN], in1=t1[:], op=OP.add
                )
                ax = epool.tile([P, D_IN], dtype=f32, tag="ax")
                nc.vector.tensor_scalar(
                    out=ax[:], in0=t2[:], scalar1=dinvT[:, g : g + 1], scalar2=None,
                    op0=OP.mult,
                )
                trp = pspool.tile([P, P], dtype=f32, tag="small")
                nc.tensor.transpose(out=trp[:], in_=ax[:], identity=ident[:])
                axT = epool.tile([P, P], dtype=f32, tag="axT")
                nc.vector.tensor_copy(axT[:], trp[:])
                o1p = pspool.tile([P, D_HID], dtype=f32, tag="mm")
                nc.tensor.matmul(
                    o1p[:], lhsT=axT[:], rhs=W1s[:], start=True, stop=True
                )
                x1a = epool.tile([P, D_HID], dtype=f32, tag="x1a")
                nc.vector.tensor_tensor(out=x1a[:], in0=o1p[:], in1=b1B[:], op=OP.add)
                x1t = epool.tile([P, D_HID], dtype=f32, tag="x1t")
                nc.scalar.activation(out=x1t[:], in_=x1a[:], func=AF.Tanh)
                nc.vector.tensor_scalar(
                    out=x1sT[:, g * D_HID : (g + 1) * D_HID], in0=x1t[:],
                    scalar1=dinvT[:, g : g + 1], scalar2=None, op0=OP.mult,
                )
                nc.sync.dma_start(
                    out=x1s_loc[g * P : (g + 1) * P, :],
                    in_=x1sT[:, g * D_HID : (g + 1) * D_HID],
                )

            if debug:
                nc.sync.dma_start(out=dbg_x1s[:, :], in_=x1sT[:])

            # ---------------- stage F: allgather x1s ----------------
            nc.gpsimd.collective_compute(
                "AllGather", OP.bypass, rg, ins=[x1s_loc[:, :]],
                outs=[x1s_full[:, :]],
            )

            # ---------------- stage G: layer 2 + skip ----------------
            for g in range(NG):
                gat2 = gpool.tile([P, F * D_HID], dtype=f32, tag="gat")
                for t in range(F):
                    col = g * F + t
                    nc.gpsimd.indirect_dma_start(
                        out=gat2[:, t * D_HID : (t + 1) * D_HID],
                        out_offset=None, in_=x1s_full[:, :],
                        in_offset=bass.IndirectOffsetOnAxis(
                            ap=src2T[:, col : col + 1], axis=0
                        ),
                    )
                agg2 = pspool.tile([P, D_HID], dtype=f32, tag="agg")
                for t in range(F):
                    col = g * F + t
                    pt = ppool.tile([P, P], dtype=f32, tag="pt")
                    nc.vector.tensor_scalar(
                        out=pt[:], in0=iota_f[:], scalar1=dstT[:, col : col + 1],
                        scalar2=wT[:, col : col + 1], op0=OP.is_equal, op1=OP.mult,
                    )
                    nc.tensor.matmul(
                        agg2[:], lhsT=pt[:],
                        rhs=gat2[:, t * D_HID : (t + 1) * D_HID],
                        start=(t == 0), stop=(t == F - 1),
                    )
                # AX2 = dinv * (agg2 + x1s_own)
                t3 = epool.tile([P, D_HID], dtype=f32, tag="t3")
                nc.vector.tensor_tensor(
                    out=t3[:], in0=agg2[:],
                    in1=x1sT[:, g * D_HID : (g + 1) * D_HID], op=OP.add,
                )
                ax2 = epool.tile([P, D_HID], dtype=f32, tag="ax2")
                nc.vector.tensor_scalar(
                    out=ax2[:], in0=t3[:], scalar1=dinvT[:, g : g + 1], scalar2=None,
                    op0=OP.mult,
                )
                o2p = pspool.tile([P, D_OUT], dtype=f32, tag="mm")
                for h in range(2):
                    trp2 = pspool.tile([P, P], dtype=f32, tag="small")
                    nc.tensor.transpose(
                        out=trp2[:], in_=ax2[:, h * P : (h + 1) * P],
                        identity=ident[:],
                    )
                    ax2T = epool.tile([P, P], dtype=f32, tag=f"ax2T{h}")
                    nc.vector.tensor_copy(ax2T[:], trp2[:])
                    nc.tensor.matmul(
                        o2p[:], lhsT=ax2T[:], rhs=(W2a[:] if h == 0 else W2b[:]),
                        start=(h == 0), stop=False,
                    )
                xwT2 = epool.tile([P, P], dtype=f32, tag="xwT2")
                nc.sync.dma_start(
                    out=xwT2[:], in_=xwinT[:, g * P : (g + 1) * P]
                )
                nc.tensor.matmul(
                    o2p[:], lhsT=xwT2[:], rhs=Wss[:], start=False, stop=True
                )
                ob = epool.tile([P, D_OUT], dtype=f32, tag="ob")
                nc.vector.tensor_tensor(out=ob[:], in0=o2p[:], in1=b2sB[:], op=OP.add)
                nc.sync.dma_start(
                    out=out_loc[g * P : (g + 1) * P, :], in_=ob[:]
                )

    nc.compile()
    return nc


# ======================================================================
# Driver
# ======================================================================

_CACHE = {}


def _get_program(NG):
    key = ("prog", NG)
    if key not in _CACHE:
        _CACHE[key] = _build_program(NG)
    return _CACHE[key]


def _run(inputs, trace=False):
    from concourse.bass_utils import run_bass_kernel_spmd

    edge_index = np.asarray(inputs["edge_index"])
    ei_key = hash(edge_index.tobytes())
    pkey = ("prep", ei_key)
    if pkey not in _CACHE:
        _CACHE[pkey] = _preprocess(edge_index)
    prep = _CACHE[pkey]
    NG = prep["NG"]

    nc = _get_program(NG)
    maps = _host_inputs(prep, inputs["edge_feats"], inputs["node_feats"])

    shared = {
        "We_r": np.asarray(inputs["We"], np.float32).reshape(1, D_EDGE),
        "be_r": np.asarray(inputs["be"], np.float32).reshape(1, 1),
        "W1": np.asarray(inputs["W1"], np.float32),
        "b1_r": np.asarray(inputs["b1"], np.float32).reshape(1, D_HID),
        "W2": np.asarray(inputs["W2"], np.float32),
        "Ws": np.asarray(inputs["Ws"], np.float32),
        "b2s_r": (
            np.asarray(inputs["b2"], np.float32) + np.asarray(inputs["bs"], np.float32)
        ).reshape(1, D_OUT),
    }
    drop = ("srcq", "sdivq")
    in_maps = [
        {**{k: v for k, v in m.items() if k not in drop}, **shared} for m in maps
    ]

    res = run_bass_kernel_spmd(
        nc, in_maps, core_ids=list(range(NCORES)), trace=trace
    )

    out = np.zeros((N_NODES, D_OUT), dtype=np.float32)
    for k, pc in enumerate(prep["per_core"]):
        ol = res.results[k]["out_loc"]
        for g, (base, nend, _, _) in enumerate(pc["groups"]):
            if nend > base:
                out[base:nend] = ol[g * P : g * P + (nend - base)]
    return out, res


def kernel(**inputs):
    out, _ = _run(inputs, trace=False)
    return out

